# revision 50
# speedup vs baseline: 1.2096x; 1.0094x over previous
"""Columnwise imputer (per-feature LSTM) Trainium2 kernel.

Problem: D=32 independent per-feature LSTMs (input D-1=31, hidden H=64),
B=128, T=128.  x_hat[b,t,d] = W_out[d] @ h_d(t) + b_out[d].

Sharding: expert-parallel over the D feature axis -- 4 features per core
(2 "pairs" of 2 features).  Each core runs its 4 LSTMs over the full
batch; the host gathers per-core [B, T, 4] outputs.

Device kernel structure (per core, per timestep t, per pair):
  - gates psum tile fp32; each [128,128] chunk: partitions 0-63 feature A
    of the pair, 64-127 feature B; free dim = batch.
  - per chunk: ip matmul (K=33: 32 features + ones row carrying the bias)
    accumulated with rec matmul (K=128: block-diag [h_A; h_B] weights).
  - ScalarE: sigmoid over i,f,o chunks, tanh over g, later tanh over c.
  - VectorE: t1 = i*g, t2 = f*c, c' = t1+t2, h' = o*tanh(c').
  - output head in-loop: y^T[b, feat] = h_t (stationary) @ W_out cols,
    N=2 matmul per pair into a persistent psum tile [128, 512].

The kernel is recurrence-latency-bound (wall ~= T x per-step chain), so the
default variant (v8) minimizes the serial chain: sigma-trick (all activations
are a single Sigmoid table; tanh expressed via sigmoid with weight folding),
fused scalar_tensor_tensor cell ops, split sigmoid + psum-bank split so the
o-gate is fully off the critical path, and software-pipelined emission (the
s_c/h tail of step t-1 interleaves with the matmuls/sigmoid of step t on
every engine queue).  Any chain decomposition of this op set (1 fat chain,
2 pair chains, finer splits) lands at 3.05-3.2us/step: per-step cost is
dominated by per-instruction fixed costs (ACT ~222cyc SBUF init, DVE
58-151cyc, ~100ns/semaphore hop), not data volume.

Variants (env KV): v1 batched, v2 per-pair, v3 sigma-trick,
v4 rotated single-sigma, v5 rotated split-sigma, v6/v8 two-bank psum
splits, v7 pair-blocked emission, v11+ pre-issued input projections.

Default v15 (340us cost-model / 389.8us for the old v8 default):
  - v11: ip matmuls depend only on the input, so they are pre-issued one
    step early into the psum accumulation group (TRN2 zeroing is lazy per
    2KB bank: ONE start per bank per step, everything else accumulates,
    the last rec matmul stops the group).  The h2 -> sigma1 chain segment
    shrinks from 6 matmuls to 3.  NOTE: interleaved per-chunk start/stop
    groups in one bank are ILLEGAL (start marks the whole bank pending-
    zero) -- that variant fails on hardware with rel err 0.8.
  - v12: both pairs' o-gate chunks share one psum tile + one sigmoid.
  - v14: STT runs at 1x on DVE while TT/TS get the 2x perf mode, so the
    two chain STTs become TTs: cell state is kept halved (ct = c/2, so
    ct = t1 + f*ct_prev is a plain add) and sc = tanh(c) comes from the
    Tanh table (same ACT func set as Sigmoid, no table reload), making
    h = sc * s_o a plain mul with h stored full-scale (recw/outw packing
    NOT doubled; only the g-gate sigma-trick doubling remains).
  - v15: chunked y drain (psum->sbuf DVE copy + DMA per 32-col block,
    8-way) overlapped with the loop, small first xmT DMA slice, and deep
    sbuf tile pools (SIGB=36, others 31): shallow pools were adding
    ~80ns/step of ring-reuse WAR serialization on top of the chain.
    NOTE: nc.gpsimd.tensor_copy passes CoreSim but fails the real axon
    compile path -- drain copies must stay on DVE.
  - t1 via the AFFINE_MUL_REDUCE custom DVE op ((in0*s0+s1)*in1): same
    194ns busy as the STT it replaces, but custom-ISA ops signal their
    semaphore ~60ns sooner after busy-end (the _read aux slice), so the
    downstream c-add starts at +35 instead of +95.
  - sigma2 split into per-pair [128,128] ops over the shared pgb tile:
    the first half clears the ACT queue before sc_p0 becomes ready.
Steady state ~2460ns/step (322.3us total), chain-latency-bound:
  sem hops+acks ~890 | sigma1 505 | cell DVE 543 | sc 292 | h2 127 |
  rec 159.  The 5 cross-engine hops (PE->ACT->DVE->ACT->DVE) are forced
  by the LSTM dataflow; merging sc into the next sigma1 is circular
  (h_{t-1} needs sc_{t-1}).  Probed and rejected: o-gate folded into
  sigma1 (v16, +23us), B-split chains (ACT op fixed costs saturate),
  gpsimd cell ops (Pool latency), fp8 rec weights (numerics), sigma1
  splits (ACT serialization).  Chain-exec floor for this decomposition.
Hardware-verified (8 axon trn2 cores): 322300ns, relative error
4.96e-3 vs the float64 reference (bf16 matmul inputs + bf16 state
dominate the error; the recurrence is contractive, it does not
compound).  Baseline v8 was 389839ns: -17.3%.

Host prep: masking, transpose to [feat, t, b], weight packing (block-diag,
zero-diagonal full-D input weights, biases as 33rd row, sigma-trick weight
doubling), bias b_out added on the host.
"""

import os
import sys

import numpy as np

try:
    import concourse  # noqa: F401  (provided by the axon boot environment)
except ImportError:
    sys.path.insert(0, "/root/.axon_site/_ro/trn_rl_repo")

import ml_dtypes

D = 32
H = 64
B = 128
T = 128
NCORES = 8
DLOC = D // NCORES  # 4 features per core
NPAIR = DLOC // 2  # 2 pairs per core

VARIANT = os.environ.get("KV", "v15")
SDT_NAME = os.environ.get("SDT", "bf16")  # sigmoid/tanh output dtype
CDT_NAME = os.environ.get("CDT", "bf16")  # cell state dtype
USE_GPSIMD = bool(int(os.environ.get("GPS", "0")))  # offload f*c to GPSIMD

BF16 = ml_dtypes.bfloat16

# gate row ranges in the torch-stacked [4H] layout (i, f, g, o)
GATE_SLICES = {
    "i": slice(0 * H, 1 * H),
    "f": slice(1 * H, 2 * H),
    "g": slice(2 * H, 3 * H),
    "o": slice(3 * H, 4 * H),
}

# psum chunk order (gate, pair):
if VARIANT == "v1":
    CHUNK_DEFS = [(g, p) for g in ("i", "f", "o", "g") for p in range(NPAIR)]
elif VARIANT == "v2":
    CHUNK_DEFS = [(g, p) for p in range(NPAIR) for g in ("i", "f", "o", "g")]
else:  # v3/v4/v5 sigma-trick layouts
    CHUNK_DEFS = [(g, p) for p in range(NPAIR) for g in ("i", "g", "f", "o")]

# v3 sigma-trick weight folding:
#   - h is stored as h/2 (STT produces (sigma(2c)-0.5)*o), so all recurrent
#     and output weights that contract h are doubled.
#   - tanh(x) = 2*sigmoid(2x) - 1: g-gate logits are doubled so a single
#     sigmoid pass covers all four gates; the -1/x2 correction happens in
#     cheap DVE tensor_scalar/STT ops (or is folded into downstream weights).
V3 = VARIANT in ("v3", "v4", "v5", "v6", "v7", "v8", "v9", "v10", "v11",
                 "v12", "v13")
# tanh-cell variants: h stored full-scale (sc via Tanh table), so only the
# g-gate logit doubling survives in the packing.
TANH_CELL = VARIANT in ("v14", "v15", "v16")

_CACHE = {}


def _build_bass():
    """Build (and cache) the Bass module. Same program for all 8 cores."""
    if "nc" in _CACHE:
        return _CACHE["nc"]

    import concourse.bacc as bacc
    import concourse.mybir as mybir
    import concourse.tile as tile

    f32 = mybir.dt.float32
    bf16 = mybir.dt.bfloat16
    sdt = bf16 if SDT_NAME == "bf16" else f32
    cdt = bf16 if CDT_NAME == "bf16" else f32
    SIG = mybir.ActivationFunctionType.Sigmoid
    TANH = mybir.ActivationFunctionType.Tanh

    nc = bacc.Bacc("TRN2", target_bir_lowering=False, debug=False, num_devices=NCORES)

    xmT_d = nc.dram_tensor("xmT", [D + 1, T * B], bf16, kind="ExternalInput").ap()
    ipw_d = nc.dram_tensor("ipw", [D + 1, 8 * 128], bf16, kind="ExternalInput").ap()
    recw_d = nc.dram_tensor("recw", [128, 8 * 128], bf16, kind="ExternalInput").ap()
    outw_d = nc.dram_tensor("outw", [128, 2 * NPAIR], bf16, kind="ExternalInput").ap()
    y_d = nc.dram_tensor("y", [B, NPAIR * 2 * T], f32, kind="ExternalOutput").ap()

    with tile.TileContext(nc) as tc:
        with (
            tc.tile_pool(name="const", bufs=1) as const_pool,
            tc.tile_pool(name="psum_gates", bufs=int(os.environ.get("PGBUFS", "2")), space="PSUM") as pg_pool,
            tc.tile_pool(name="psum_y", bufs=1, space="PSUM") as py_pool,
            tc.tile_pool(name="sig", bufs=int(os.environ.get(
                "SIGB", os.environ.get("SBUFS", "36")))) as sig_pool,
            tc.tile_pool(name="tan", bufs=int(os.environ.get(
                "TANB", os.environ.get("SBUFS", "31")))) as tan_pool,
            tc.tile_pool(name="tmp", bufs=int(os.environ.get(
                "TMPB", os.environ.get("SBUFS", "31")))) as tmp_pool,
            tc.tile_pool(name="cst", bufs=int(os.environ.get(
                "CSTB", os.environ.get("SBUFS", "31")))) as c_pool,
            tc.tile_pool(name="hst", bufs=int(os.environ.get(
                "HSTB", os.environ.get("SBUFS", "31")))) as h_pool,
        ):
            xmT = const_pool.tile([D + 1, T * B], bf16)
            ipw = const_pool.tile([D + 1, 8 * 128], bf16)
            recw = const_pool.tile([128, 8 * 128], bf16)
            outw = const_pool.tile([128, 2 * NPAIR], bf16)
            # iteration 0 needs only ipw + xmT for t=0,1 (ip pre-issue), so a
            # tiny first slice unblocks the first matmuls; recw/outw are
            # first consumed at t=1, the remaining xmT chunks later still.
            nc.sync.dma_start(out=ipw, in_=ipw_d)
            nc.sync.dma_start(out=xmT[:, 0 : 2 * B], in_=xmT_d[:, 0 : 2 * B])
            nc.sync.dma_start(out=recw, in_=recw_d)
            nc.sync.dma_start(out=outw, in_=outw_d)
            bounds = [2 * B] + [i * (T * B) // 8 for i in range(1, 9)]
            for lo, hi in zip(bounds[:-1], bounds[1:]):
                nc.sync.dma_start(out=xmT[:, lo:hi], in_=xmT_d[:, lo:hi])

            y_ps = py_pool.tile([B, NPAIR * 2 * T], f32)

            if VARIANT in ("v6", "v8", "v9", "v10", "v11", "v12", "v13",
                           "v14", "v15", "v16"):
                pgb_bufs = int(os.environ.get(
                    "PGB", "2" if VARIANT in ("v12", "v13", "v14", "v15") else "1"))
                with tc.tile_pool(name="psum_b", bufs=pgb_bufs, space="PSUM") as pgb_pool:
                    emit2 = {"v6": _emit_v6, "v8": _emit_v8, "v9": _emit_v9,
                             "v10": _emit_v10, "v11": _emit_v11,
                             "v12": _emit_v12, "v13": _emit_v13,
                             "v14": _emit_v14, "v15": _emit_v15,
                             "v16": _emit_v16}[VARIANT]
                    kw = {}
                    if VARIANT in ("v15", "v16"):
                        y_sb15 = const_pool.tile([B, NPAIR * 2 * T], f32)
                        kw = {"y_d": (y_d, y_sb15)}
                    emit2(nc, tc, mybir, pg_pool, pgb_pool, sig_pool,
                          tan_pool, tmp_pool, c_pool, h_pool, xmT, ipw,
                          recw, outw, y_ps, f32, sdt, cdt, SIG, TANH, **kw)
            else:
                emit = {"v1": _emit_v1, "v2": _emit_v2, "v3": _emit_v3,
                        "v4": _emit_v4, "v5": _emit_v5, "v7": _emit_v7}[VARIANT]
                emit(nc, tc, mybir, pg_pool, sig_pool, tan_pool, tmp_pool,
                     c_pool, h_pool, xmT, ipw, recw, outw, y_ps, f32, sdt, cdt,
                     SIG, TANH)

            if VARIANT != "v15":
                y_sb = const_pool.tile([B, NPAIR * 2 * T], f32)
                nc.vector.tensor_copy(y_sb, y_ps)
                nc.sync.dma_start(out=y_d, in_=y_sb)

    nc.compile()
    _CACHE["nc"] = nc
    return nc


def _emit_v1(nc, tc, mybir, pg_pool, sig_pool, tan_pool, tmp_pool, c_pool,
             h_pool, xmT, ipw, recw, outw, y_ps, f32, sdt, cdt, SIG, TANH):
    """Batched layout: psum [i01|i23|f01|f23|o01|o23|g01|g23]."""
    h_prev = None
    c_prev = None
    for t in range(T):
        pg = pg_pool.tile([128, 8 * 128], f32, tag="pg")
        xm_t = xmT[:, t * B : (t + 1) * B]
        for ci in range(8):
            _, pair = CHUNK_DEFS[ci]
            sl = slice(ci * 128, (ci + 1) * 128)
            nc.tensor.matmul(pg[:, sl], ipw[:, sl], xm_t, start=True, stop=(t == 0))
            if t > 0:
                nc.tensor.matmul(
                    pg[:, sl], recw[:, sl], h_prev[:, pair, :],
                    start=False, stop=True,
                )

        ssig = sig_pool.tile([128, 6 * 128], sdt, tag="ssig")
        nc.scalar.activation(ssig, pg[:, 0 : 6 * 128], SIG)
        stan = tan_pool.tile([128, 2 * 128], sdt, tag="stan")
        nc.scalar.activation(stan, pg[:, 6 * 128 : 8 * 128], TANH)

        i_ap = ssig[:, 0:256]
        f_ap = ssig[:, 256:512]
        o_ap = ssig[:, 512:768]

        c_new = c_pool.tile([128, 2 * 128], cdt, tag="c")
        if t == 0:
            nc.vector.tensor_mul(c_new, i_ap, stan)
        else:
            t1 = tmp_pool.tile([128, 2 * 128], sdt, tag="t1")
            nc.vector.tensor_mul(t1, i_ap, stan)
            t2 = tmp_pool.tile([128, 2 * 128], cdt, tag="t2")
            nc.vector.tensor_mul(t2, f_ap, c_prev)
            nc.vector.tensor_add(c_new, t1, t2)

        sc = tan_pool.tile([128, 2 * 128], sdt, tag="sc")
        nc.scalar.activation(sc, c_new, TANH)

        h_new = h_pool.tile([128, NPAIR, B], mybir.dt.bfloat16, tag="h")
        nc.vector.tensor_mul(
            h_new,
            o_ap.rearrange("p (q b) -> p q b", q=NPAIR),
            sc.rearrange("p (q b) -> p q b", q=NPAIR),
        )

        for pair in range(NPAIR):
            nc.tensor.matmul(
                y_ps[:, pair * 2 * T + 2 * t : pair * 2 * T + 2 * t + 2],
                h_new[:, pair, :],
                outw[:, 2 * pair : 2 * pair + 2],
                start=True, stop=True,
            )

        h_prev = h_new
        c_prev = c_new


def _emit_v2(nc, tc, mybir, pg_pool, sig_pool, tan_pool, tmp_pool, c_pool,
             h_pool, xmT, ipw, recw, outw, y_ps, f32, sdt, cdt, SIG, TANH):
    """Per-pair chains: psum per (t, pair) = [i|f|o|g], chunks at
    ipw/recw columns (pair*4 + k)*128."""
    h_prev = [None] * NPAIR
    c_prev = [None] * NPAIR
    for t in range(T):
        xm_t = xmT[:, t * B : (t + 1) * B]
        for pair in range(NPAIR):
            pg = pg_pool.tile([128, 4 * 128], f32, tag=f"pg{pair}")
            for k in range(4):
                ci = pair * 4 + k
                wsl = slice(ci * 128, (ci + 1) * 128)
                psl = slice(k * 128, (k + 1) * 128)
                nc.tensor.matmul(
                    pg[:, psl], ipw[:, wsl], xm_t, start=True, stop=(t == 0)
                )
                if t > 0:
                    nc.tensor.matmul(
                        pg[:, psl], recw[:, wsl], h_prev[pair],
                        start=False, stop=True,
                    )

            ssig = sig_pool.tile([128, 3 * 128], sdt, tag=f"ssig{pair}")
            nc.scalar.activation(ssig, pg[:, 0 : 3 * 128], SIG)
            stan = tan_pool.tile([128, 128], sdt, tag=f"stan{pair}")
            nc.scalar.activation(stan, pg[:, 3 * 128 : 4 * 128], TANH)

            i_ap = ssig[:, 0:128]
            f_ap = ssig[:, 128:256]
            o_ap = ssig[:, 256:384]

            c_new = c_pool.tile([128, 128], cdt, tag=f"c{pair}")
            if t == 0:
                nc.vector.tensor_mul(c_new, i_ap, stan)
            else:
                t1 = tmp_pool.tile([128, 128], sdt, tag=f"t1{pair}")
                nc.vector.tensor_mul(t1, i_ap, stan)
                t2 = tmp_pool.tile([128, 128], cdt, tag=f"t2{pair}")
                nc.vector.tensor_mul(t2, f_ap, c_prev[pair])
                nc.vector.tensor_add(c_new, t1, t2)

            sc = tan_pool.tile([128, 128], sdt, tag=f"sc{pair}")
            nc.scalar.activation(sc, c_new, TANH)

            h_new = h_pool.tile([128, B], mybir.dt.bfloat16, tag=f"h{pair}")
            nc.vector.tensor_mul(h_new, o_ap, sc)

            nc.tensor.matmul(
                y_ps[:, pair * 2 * T + 2 * t : pair * 2 * T + 2 * t + 2],
                h_new,
                outw[:, 2 * pair : 2 * pair + 2],
                start=True, stop=True,
            )

            h_prev[pair] = h_new
            c_prev[pair] = c_new


def _emit_v3(nc, tc, mybir, pg_pool, sig_pool, tan_pool, tmp_pool, c_pool,
             h_pool, xmT, ipw, recw, outw, y_ps, f32, sdt, cdt, SIG, TANH):
    """Sigma-trick + chain-latency-optimized emission.

    Chunk order per pair is [i, g, f, o] (see CHUNK_DEFS).  Per pair per t:
      sigma1 = sigmoid(pg[i,g])   (after only the first 4 matmuls)
      t1'   = (s_g - 0.5) * s_i   (STT; == i*g/2)
      sigma2 = sigmoid(pg[f,o])   (off critical path, overlaps DVE)
      t2    = s_f * c_prev        (TT)
      c     = 2*t1' + t2          (STT)
      s_c   = sigmoid(2c) fp32
      h2    = (s_c - 0.5) * s_o   (STT; == h/2, x2 folded into recw/outw)
    """
    ALU = mybir.AluOpType
    h_prev = [None] * NPAIR
    c_prev = [None] * NPAIR
    y_mm = [None] * NPAIR  # deferred y matmul args from previous t

    for t in range(T):
        xm_t = xmT[:, t * B : (t + 1) * B]
        pgs = [None] * NPAIR

        # 1) gate matmuls for t: per chunk [rec(start), ip(stop)] -- PSUM
        # allows only one open accumulation group per bank.
        for pair in range(NPAIR):
            pg = pgs[pair] = pg_pool.tile(
                [128, 4 * 128], f32, tag=f"pg{pair}", name=f"pg{pair}"
            )
            for k in range(4):
                ci = pair * 4 + k
                sl = slice(k * 128, (k + 1) * 128)
                wsl = slice(ci * 128, (ci + 1) * 128)
                if t > 0:
                    nc.tensor.matmul(
                        pg[:, sl], recw[:, wsl], h_prev[pair],
                        start=True, stop=False,
                    )
                nc.tensor.matmul(
                    pg[:, sl], ipw[:, wsl], xm_t,
                    start=(t == 0), stop=True,
                )
        # y matmuls for t-1 (PE; operands long ready)
        for pair in range(NPAIR):
            if y_mm[pair] is not None:
                out_sl, h_tile = y_mm[pair]
                nc.tensor.matmul(
                    y_ps[:, out_sl], h_tile, outw[:, 2 * pair : 2 * pair + 2],
                    start=True, stop=True,
                )
                y_mm[pair] = None

        # 2) sigma1 over [i, g] chunks (critical path: t1')
        sig1 = []
        for pair in range(NPAIR):
            s1 = sig_pool.tile([128, 2 * 128], sdt, tag=f"s1{pair}")
            nc.scalar.activation(s1, pgs[pair][:, 0:256], SIG)
            sig1.append(s1)

        # 3) t1' = (s_g - 0.5) * s_i  == i*g/2   (STT)
        t1s = []
        for pair in range(NPAIR):
            t1 = tmp_pool.tile([128, 128], sdt, tag=f"t1{pair}")
            nc.vector.scalar_tensor_tensor(
                t1, sig1[pair][:, 128:256], 0.5, sig1[pair][:, 0:128],
                ALU.subtract, ALU.mult,
            )
            t1s.append(t1)

        # 4) sigma2 over [f, o] chunks (overlaps DVE work above)
        sig2 = []
        for pair in range(NPAIR):
            s2 = sig_pool.tile([128, 2 * 128], sdt, tag=f"s2{pair}")
            nc.scalar.activation(s2, pgs[pair][:, 256:512], SIG)
            sig2.append(s2)

        # 5) t2 = f * c_prev ;  6) c = 2*t1' + t2
        for pair in range(NPAIR):
            c_new = c_pool.tile([128, 128], cdt, tag=f"c{pair}")
            if t == 0:
                nc.vector.tensor_scalar_mul(c_new, t1s[pair], 2.0)
            else:
                t2 = tmp_pool.tile([128, 128], cdt, tag=f"t2{pair}")
                eng = nc.gpsimd if USE_GPSIMD else nc.vector
                eng.tensor_mul(t2, sig2[pair][:, 0:128], c_prev[pair])
                nc.vector.scalar_tensor_tensor(
                    c_new, t1s[pair], 2.0, t2, ALU.mult, ALU.add
                )
            c_prev[pair] = c_new

        # 7) s_c = sigmoid(2c)  (fp32 out: avoids cancellation in s_c-0.5)
        # 8) h2 = (s_c - 0.5) * o   == h/2
        for pair in range(NPAIR):
            sc = tan_pool.tile([128, 128], f32, tag=f"sc{pair}")
            nc.scalar.activation(sc, c_prev[pair], SIG, scale=2.0)
            h2 = h_pool.tile([128, B], mybir.dt.bfloat16, tag=f"h{pair}")
            nc.vector.scalar_tensor_tensor(
                h2, sc, 0.5, sig2[pair][:, 128:256],
                ALU.subtract, ALU.mult,
            )
            h_prev[pair] = h2
            y_mm[pair] = (
                slice(pair * 2 * T + 2 * t, pair * 2 * T + 2 * t + 2),
                h2,
            )

    # trailing y matmuls for t = T-1
    for pair in range(NPAIR):
        out_sl, h_tile = y_mm[pair]
        nc.tensor.matmul(
            y_ps[:, out_sl], h_tile, outw[:, 2 * pair : 2 * pair + 2],
            start=True, stop=True,
        )


def _emit_v4(*args, **kw):
    _emit_rotated(*args, split_sigma=False, **kw)


def _emit_v5(*args, **kw):
    _emit_rotated(*args, split_sigma=True, **kw)


def _emit_rotated(nc, tc, mybir, pg_pool, sig_pool, tan_pool, tmp_pool, c_pool,
                  h_pool, xmT, ipw, recw, outw, y_ps, f32, sdt, cdt, SIG, TANH,
                  split_sigma=False):
    """Software-pipelined emission: iteration tau emits the *tail* of step
    tau-1 (s_c, h2) before the matmuls/sigmoid/cell ops of step tau, so each
    engine's in-order queue cycles through both pair-chains with the tail of
    one step overlapping the head of the next.

    Chunk order [i, g, f, o].  split_sigma: sigma1=[i,g,f], sigma2=[o]
    (sigma2 is only needed by h2 one iteration later)."""
    ALU = mybir.AluOpType
    h_prev = [None] * NPAIR
    c_prev = [None] * NPAIR
    o_src = [None] * NPAIR  # AP of sigmoid(o) for the h2 of the previous step
    y_mm = [None] * NPAIR

    for t in range(T + 1):
        # ---- tail of step t-1: s_c, h2 ----
        if t > 0:
            scs = []
            for pair in range(NPAIR):
                sc = tan_pool.tile([128, 128], f32, tag=f"sc{pair}", name=f"sc{pair}")
                nc.scalar.activation(sc, c_prev[pair], SIG, scale=2.0)
                scs.append(sc)
            for pair in range(NPAIR):
                h2 = h_pool.tile([128, B], mybir.dt.bfloat16, tag=f"h{pair}",
                                 name=f"h{pair}")
                nc.vector.scalar_tensor_tensor(
                    h2, scs[pair], 0.5, o_src[pair], ALU.subtract, ALU.mult
                )
                h_prev[pair] = h2
                y_mm[pair] = (
                    slice(pair * 2 * T + 2 * (t - 1), pair * 2 * T + 2 * (t - 1) + 2),
                    h2,
                )
        if t == T:
            break

        xm_t = xmT[:, t * B : (t + 1) * B]

        # ---- gate matmuls for t ----
        pgs = []
        for pair in range(NPAIR):
            pg = pg_pool.tile([128, 4 * 128], f32, tag=f"pg{pair}", name=f"pg{pair}")
            pgs.append(pg)
            for k in range(4):
                ci = pair * 4 + k
                sl = slice(k * 128, (k + 1) * 128)
                wsl = slice(ci * 128, (ci + 1) * 128)
                if t > 0:
                    nc.tensor.matmul(
                        pg[:, sl], recw[:, wsl], h_prev[pair],
                        start=True, stop=False,
                    )
                nc.tensor.matmul(
                    pg[:, sl], ipw[:, wsl], xm_t, start=(t == 0), stop=True
                )
        # y matmuls for t-1
        for pair in range(NPAIR):
            if y_mm[pair] is not None:
                out_sl, h_tile = y_mm[pair]
                nc.tensor.matmul(
                    y_ps[:, out_sl], h_tile, outw[:, 2 * pair : 2 * pair + 2],
                    start=True, stop=True,
                )
                y_mm[pair] = None

        # ---- sigmoid(s) for t ----
        sigs = []
        if split_sigma:
            for pair in range(NPAIR):
                s1 = sig_pool.tile([128, 3 * 128], sdt, tag=f"s1{pair}",
                                   name=f"s1{pair}")
                nc.scalar.activation(s1, pgs[pair][:, 0:384], SIG)
                sigs.append(s1)
            for pair in range(NPAIR):
                s2 = sig_pool.tile([128, 128], sdt, tag=f"s2{pair}",
                                   name=f"s2{pair}")
                nc.scalar.activation(s2, pgs[pair][:, 384:512], SIG)
                o_src[pair] = s2
        else:
            for pair in range(NPAIR):
                s = sig_pool.tile([128, 4 * 128], sdt, tag=f"s{pair}",
                                  name=f"s{pair}")
                nc.scalar.activation(s, pgs[pair], SIG)
                sigs.append(s)
                o_src[pair] = s[:, 384:512]

        # ---- cell update for t: t1' = (s_g-0.5)*s_i ; c = 2*t1' + f*c ----
        for pair in range(NPAIR):
            s = sigs[pair]
            t1 = tmp_pool.tile([128, 128], sdt, tag=f"t1{pair}", name=f"t1{pair}")
            nc.vector.scalar_tensor_tensor(
                t1, s[:, 128:256], 0.5, s[:, 0:128], ALU.subtract, ALU.mult
            )
            c_new = c_pool.tile([128, 128], cdt, tag=f"c{pair}", name=f"c{pair}")
            if t == 0:
                nc.vector.tensor_scalar_mul(c_new, t1, 2.0)
            else:
                t2 = tmp_pool.tile([128, 128], cdt, tag=f"t2{pair}", name=f"t2{pair}")
                eng = nc.gpsimd if USE_GPSIMD else nc.vector
                eng.tensor_mul(t2, s[:, 256:384], c_prev[pair])
                nc.vector.scalar_tensor_tensor(
                    c_new, t1, 2.0, t2, ALU.mult, ALU.add
                )
            c_prev[pair] = c_new

    # trailing y matmuls for t = T-1
    for pair in range(NPAIR):
        out_sl, h_tile = y_mm[pair]
        nc.tensor.matmul(
            y_ps[:, out_sl], h_tile, outw[:, 2 * pair : 2 * pair + 2],
            start=True, stop=True,
        )


def _emit_v7(nc, tc, mybir, pg_pool, sig_pool, tan_pool, tmp_pool, c_pool,
             h_pool, xmT, ipw, recw, outw, y_ps, f32, sdt, cdt, SIG, TANH):
    """v5 chain ops, but emitted as complete per-pair blocks so each
    engine's in-order queue alternates whole chain-stages of the two pairs
    (anti-phase) instead of interleaving the same stage of both pairs."""
    ALU = mybir.AluOpType
    h_prev = [None] * NPAIR
    c_prev = [None] * NPAIR
    o_src = [None] * NPAIR
    y_mm = [None] * NPAIR

    for t in range(T + 1):
        for pair in range(NPAIR):
            # ---- tail of step t-1 for this pair ----
            if t > 0:
                sc = tan_pool.tile([128, 128], f32, tag=f"sc{pair}", name=f"sc{pair}")
                nc.scalar.activation(sc, c_prev[pair], SIG, scale=2.0)
                h2 = h_pool.tile([128, B], mybir.dt.bfloat16, tag=f"h{pair}",
                                 name=f"h{pair}")
                nc.vector.scalar_tensor_tensor(
                    h2, sc, 0.5, o_src[pair], ALU.subtract, ALU.mult
                )
                h_prev[pair] = h2
            if t == T:
                continue

            xm_t = xmT[:, t * B : (t + 1) * B]
            pg = pg_pool.tile([128, 4 * 128], f32, tag=f"pg{pair}", name=f"pg{pair}")
            for k in range(4):
                ci = pair * 4 + k
                sl = slice(k * 128, (k + 1) * 128)
                wsl = slice(ci * 128, (ci + 1) * 128)
                if t > 0:
                    nc.tensor.matmul(pg[:, sl], recw[:, wsl], h_prev[pair],
                                     start=True, stop=False)
                nc.tensor.matmul(pg[:, sl], ipw[:, wsl], xm_t,
                                 start=(t == 0), stop=True)
            # y matmul for t-1 of this pair
            if y_mm[pair] is not None:
                out_sl, h_tile = y_mm[pair]
                nc.tensor.matmul(
                    y_ps[:, out_sl], h_tile, outw[:, 2 * pair : 2 * pair + 2],
                    start=True, stop=True,
                )
            if t > 0:
                y_mm[pair] = (
                    slice(pair * 2 * T + 2 * (t - 1), pair * 2 * T + 2 * (t - 1) + 2),
                    h_prev[pair],
                )

            # sigma1 = [i, g, f]; sigma2 = [o]
            s1 = sig_pool.tile([128, 3 * 128], sdt, tag=f"s1{pair}", name=f"s1{pair}")
            nc.scalar.activation(s1, pg[:, 0:384], SIG)
            s2 = sig_pool.tile([128, 128], sdt, tag=f"s2{pair}", name=f"s2{pair}")
            nc.scalar.activation(s2, pg[:, 384:512], SIG)
            o_src[pair] = s2

            # cell update
            t1 = tmp_pool.tile([128, 128], sdt, tag=f"t1{pair}", name=f"t1{pair}")
            nc.vector.scalar_tensor_tensor(
                t1, s1[:, 128:256], 0.5, s1[:, 0:128], ALU.subtract, ALU.mult
            )
            c_new = c_pool.tile([128, 128], cdt, tag=f"c{pair}", name=f"c{pair}")
            if t == 0:
                nc.vector.tensor_scalar_mul(c_new, t1, 2.0)
            else:
                t2 = tmp_pool.tile([128, 128], cdt, tag=f"t2{pair}", name=f"t2{pair}")
                nc.vector.tensor_mul(t2, s1[:, 256:384], c_prev[pair])
                nc.vector.scalar_tensor_tensor(
                    c_new, t1, 2.0, t2, ALU.mult, ALU.add
                )
            c_prev[pair] = c_new

    # trailing y matmuls: the still-pending (T-2) and the final (T-1)
    for pair in range(NPAIR):
        if y_mm[pair] is not None:
            out_sl, h_tile = y_mm[pair]
            nc.tensor.matmul(
                y_ps[:, out_sl], h_tile, outw[:, 2 * pair : 2 * pair + 2],
                start=True, stop=True,
            )
        nc.tensor.matmul(
            y_ps[:, pair * 2 * T + 2 * (T - 1) : pair * 2 * T + 2 * (T - 1) + 2],
            h_prev[pair], outw[:, 2 * pair : 2 * pair + 2],
            start=True, stop=True,
        )


def _emit_v8(nc, tc, mybir, pga_pool, pgb_pool, sig_pool, tan_pool, tmp_pool,
             c_pool, h_pool, xmT, ipw, recw, outw, y_ps, f32, sdt, cdt,
             SIG, TANH):
    """v5 + bank split matching the sigma split: pga=[i,g,f] (bufs=2, the
    sigma1 bank -- sigma1 now waits only 6 matmuls), pgb=[o] (bufs=1,
    sigma2 is fully off the critical path)."""
    ALU = mybir.AluOpType
    h_prev = [None] * NPAIR
    c_prev = [None] * NPAIR
    o_src = [None] * NPAIR
    y_mm = [None] * NPAIR

    for t in range(T + 1):
        if t > 0:
            scs = []
            for pair in range(NPAIR):
                sc = tan_pool.tile([128, 128], f32, tag=f"sc{pair}", name=f"sc{pair}")
                nc.scalar.activation(sc, c_prev[pair], SIG, scale=2.0)
                scs.append(sc)
            for pair in range(NPAIR):
                h2 = h_pool.tile([128, B], mybir.dt.bfloat16, tag=f"h{pair}",
                                 name=f"h{pair}")
                nc.vector.scalar_tensor_tensor(
                    h2, scs[pair], 0.5, o_src[pair], ALU.subtract, ALU.mult
                )
                h_prev[pair] = h2
                y_mm[pair] = (
                    slice(pair * 2 * T + 2 * (t - 1), pair * 2 * T + 2 * (t - 1) + 2),
                    h2,
                )
        if t == T:
            break

        xm_t = xmT[:, t * B : (t + 1) * B]

        # [i, g, f] chunks into pga (both pairs), then [o] into pgb
        pgas, pgbs = [], []
        for pair in range(NPAIR):
            pga = pga_pool.tile([128, 3 * 128], f32, tag=f"pga{pair}",
                                name=f"pga{pair}")
            pgas.append(pga)
            for k in range(3):
                ci = pair * 4 + k
                sl = slice(k * 128, (k + 1) * 128)
                wsl = slice(ci * 128, (ci + 1) * 128)
                if t > 0:
                    nc.tensor.matmul(pga[:, sl], recw[:, wsl], h_prev[pair],
                                     start=True, stop=False)
                nc.tensor.matmul(pga[:, sl], ipw[:, wsl], xm_t,
                                 start=(t == 0), stop=True)
        for pair in range(NPAIR):
            pgb = pgb_pool.tile([128, 128], f32, tag=f"pgb{pair}",
                                name=f"pgb{pair}")
            pgbs.append(pgb)
            ci = pair * 4 + 3
            wsl = slice(ci * 128, (ci + 1) * 128)
            if t > 0:
                nc.tensor.matmul(pgb, recw[:, wsl], h_prev[pair],
                                 start=True, stop=False)
            nc.tensor.matmul(pgb, ipw[:, wsl], xm_t,
                             start=(t == 0), stop=True)
        for pair in range(NPAIR):
            if y_mm[pair] is not None:
                out_sl, h_tile = y_mm[pair]
                nc.tensor.matmul(
                    y_ps[:, out_sl], h_tile, outw[:, 2 * pair : 2 * pair + 2],
                    start=True, stop=True,
                )
                y_mm[pair] = None

        # sigma1 = [i, g, f] (chain); sigma2 = [o] (off-chain)
        s1s = []
        for pair in range(NPAIR):
            s1 = sig_pool.tile([128, 3 * 128], sdt, tag=f"s1{pair}", name=f"s1{pair}")
            nc.scalar.activation(s1, pgas[pair], SIG)
            s1s.append(s1)
        for pair in range(NPAIR):
            s2 = sig_pool.tile([128, 128], sdt, tag=f"s2{pair}", name=f"s2{pair}")
            nc.scalar.activation(s2, pgbs[pair], SIG)
            o_src[pair] = s2

        for pair in range(NPAIR):
            s1 = s1s[pair]
            t1 = tmp_pool.tile([128, 128], sdt, tag=f"t1{pair}", name=f"t1{pair}")
            nc.vector.scalar_tensor_tensor(
                t1, s1[:, 128:256], 0.5, s1[:, 0:128], ALU.subtract, ALU.mult
            )
            c_new = c_pool.tile([128, 128], cdt, tag=f"c{pair}", name=f"c{pair}")
            if t == 0:
                nc.vector.tensor_scalar_mul(c_new, t1, 2.0)
            else:
                t2 = tmp_pool.tile([128, 128], cdt, tag=f"t2{pair}", name=f"t2{pair}")
                nc.vector.tensor_mul(t2, s1[:, 256:384], c_prev[pair])
                nc.vector.scalar_tensor_tensor(
                    c_new, t1, 2.0, t2, ALU.mult, ALU.add
                )
            c_prev[pair] = c_new

    for pair in range(NPAIR):
        out_sl, h_tile = y_mm[pair]
        nc.tensor.matmul(
            y_ps[:, out_sl], h_tile, outw[:, 2 * pair : 2 * pair + 2],
            start=True, stop=True,
        )


def _emit_v11(nc, tc, mybir, pga_pool, pgb_pool, sig_pool, tan_pool, tmp_pool,
              c_pool, h_pool, xmT, ipw, recw, outw, y_ps, f32, sdt, cdt,
              SIG, TANH, merge_o=False, gps_t2=False, tanh_cell=False,
              y_d=None):
    """v8 + input-projection matmuls hoisted off the critical path.

    The ip matmuls depend only on xm (available from t=0), so they are
    pre-issued one step early with start=True, leaving the accumulation
    group open; the rec matmuls close it (start=False, stop=True) once
    h_{t-1} lands.  The h2 -> sigma1 chain segment shrinks from 6 to 3
    matmuls.  merge_o: both pairs' o chunks share one psum tile and one
    sigmoid (fewer ACT fixed costs; frees a psum bank for bufs=2 pgb).
    gps_t2: t2 = s_f * c_prev on GPSIMD, off the DVE queue.
    tanh_cell: STT->TT on the chain DVE ops (STT runs 1x, TT gets the 2x
    perf mode): keep cell state halved (ct = c/2) so ct = t1 + t2t is a
    plain add, and compute sc = tanh(c) via the Tanh table (same ACT
    func set as Sigmoid, so no table reload) so h = sc * s_o is a plain
    mul with h stored full-scale (recw/outw packing NOT doubled)."""
    ALU = mybir.AluOpType
    h_prev = [None] * NPAIR
    c_prev = [None] * NPAIR
    o_src = [None] * NPAIR
    y_mm = [None] * NPAIR

    def alloc_pga(t):
        return [pga_pool.tile([128, 3 * 128], f32, tag=f"pga{p}",
                              name=f"pga{p}_{t}") for p in range(NPAIR)]

    def alloc_pgb(t):
        if merge_o:
            return pgb_pool.tile([128, NPAIR * 128], f32, tag="pgb",
                                 name=f"pgb_{t}")
        return [pgb_pool.tile([128, 128], f32, tag=f"pgb{p}",
                              name=f"pgb{p}_{t}") for p in range(NPAIR)]

    def ip_mms(pgas, pgb, t, close):
        # TRN2 psum zeroing is lazy per 2KB bank: start=True marks the whole
        # bank pending-zero, later writes to pending bytes overwrite (others
        # accumulate).  So each bank gets ONE group per step: the first ip
        # chunk starts it, everything else accumulates, the last rec (or, at
        # t=0 where there is no rec, the last ip) stops it.
        xm_t = xmT[:, t * B : (t + 1) * B]
        for pair in range(NPAIR):
            for k in range(3):
                ci = pair * 4 + k
                nc.tensor.matmul(
                    pgas[pair][:, k * 128 : (k + 1) * 128],
                    ipw[:, ci * 128 : (ci + 1) * 128], xm_t,
                    start=(k == 0), stop=(close and k == 2),
                )
        for pair in range(NPAIR):
            ci = pair * 4 + 3
            dst = pgb[:, pair * 128 : (pair + 1) * 128] if merge_o else pgb[pair]
            nc.tensor.matmul(dst, ipw[:, ci * 128 : (ci + 1) * 128], xm_t,
                             start=(pair == 0 or not merge_o),
                             stop=(close and (pair == NPAIR - 1 or not merge_o)))

    # prologue: gates for t=0 are ip-only (h_{-1} = 0)
    pgas_cur = alloc_pga(0)
    pgb_cur = alloc_pgb(0)
    ip_mms(pgas_cur, pgb_cur, 0, close=True)

    for t in range(T + 1):
        if t > 0:
            scs = []
            for pair in range(NPAIR):
                if tanh_cell:
                    sc = tan_pool.tile([128, 128], sdt, tag=f"sc{pair}",
                                       name=f"sc{pair}")
                    nc.scalar.activation(sc, c_prev[pair], TANH, scale=2.0)
                else:
                    sc = tan_pool.tile([128, 128], f32, tag=f"sc{pair}",
                                       name=f"sc{pair}")
                    nc.scalar.activation(sc, c_prev[pair], SIG, scale=2.0)
                scs.append(sc)
            for pair in range(NPAIR):
                h2 = h_pool.tile([128, B], mybir.dt.bfloat16, tag=f"h{pair}",
                                 name=f"h{pair}")
                if tanh_cell:
                    nc.vector.tensor_mul(h2, scs[pair], o_src[pair])
                else:
                    nc.vector.scalar_tensor_tensor(
                        h2, scs[pair], 0.5, o_src[pair], ALU.subtract, ALU.mult
                    )
                h_prev[pair] = h2
                y_mm[pair] = (
                    slice(pair * 2 * T + 2 * (t - 1), pair * 2 * T + 2 * (t - 1) + 2),
                    h2,
                )
        if t == T:
            break

        # rec matmuls accumulate into the pre-opened groups for t; the last
        # chunk per bank closes the group
        if t > 0:
            for pair in range(NPAIR):
                for k in range(3):
                    ci = pair * 4 + k
                    nc.tensor.matmul(
                        pgas_cur[pair][:, k * 128 : (k + 1) * 128],
                        recw[:, ci * 128 : (ci + 1) * 128], h_prev[pair],
                        start=False, stop=(k == 2),
                    )
            for pair in range(NPAIR):
                ci = pair * 4 + 3
                dst = (pgb_cur[:, pair * 128 : (pair + 1) * 128] if merge_o
                       else pgb_cur[pair])
                nc.tensor.matmul(dst, recw[:, ci * 128 : (ci + 1) * 128],
                                 h_prev[pair], start=False,
                                 stop=(pair == NPAIR - 1 or not merge_o))
        for pair in range(NPAIR):
            if y_mm[pair] is not None:
                out_sl, h_tile = y_mm[pair]
                nc.tensor.matmul(
                    y_ps[:, out_sl], h_tile, outw[:, 2 * pair : 2 * pair + 2],
                    start=True, stop=True,
                )
                y_mm[pair] = None
        # pre-issue ip matmuls for t+1 (group left open; rec closes it)
        if t + 1 < T:
            pgas_nxt = alloc_pga(t + 1)
            pgb_nxt = alloc_pgb(t + 1)
            ip_mms(pgas_nxt, pgb_nxt, t + 1, close=False)
        else:
            pgas_nxt = pgb_nxt = None

        # sigma1 = [i, g, f] (chain); sigma2 = [o] (off-chain)
        s1s = []
        for pair in range(NPAIR):
            s1 = sig_pool.tile([128, 3 * 128], sdt, tag=f"s1{pair}", name=f"s1{pair}")
            nc.scalar.activation(s1, pgas_cur[pair], SIG)
            s1s.append(s1)
        if merge_o:
            # one pgb psum tile, but per-pair sigmoid ops: the first half
            # clears the ACT queue sooner so sc_p0 is never queue-blocked
            s2 = sig_pool.tile([128, NPAIR * 128], sdt, tag="s2", name="s2")
            for pair in range(NPAIR):
                psl = slice(pair * 128, (pair + 1) * 128)
                nc.scalar.activation(s2[:, psl], pgb_cur[:, psl], SIG)
                o_src[pair] = s2[:, psl]
        else:
            for pair in range(NPAIR):
                s2 = sig_pool.tile([128, 128], sdt, tag=f"s2{pair}", name=f"s2{pair}")
                nc.scalar.activation(s2, pgb_cur[pair], SIG)
                o_src[pair] = s2

        for pair in range(NPAIR):
            s1 = s1s[pair]
            c_new = c_pool.tile([128, 128], cdt, tag=f"c{pair}", name=f"c{pair}")
            if tanh_cell:
                # ct = c/2 throughout: t1 = (s_g - 0.5)*s_i = i*g/2 via
                # AFFINE_MUL_REDUCE (~134ns, near-2x; plain STT runs 1x at
                # 194ns), t2t = f*ct (TT), ct_new = t1 + t2t (TT, 2x).
                if t > 0:
                    t2 = tmp_pool.tile([128, 128], cdt, tag=f"t2{pair}",
                                       name=f"t2{pair}")
                    eng = nc.gpsimd if gps_t2 else nc.vector
                    eng.tensor_mul(t2, s1[:, 256:384], c_prev[pair])
                t1 = tmp_pool.tile([128, 128], sdt, tag=f"t1{pair}", name=f"t1{pair}")
                t1a = tmp_pool.tile([128, 1], f32, tag=f"t1a{pair}",
                                    name=f"t1a{pair}")
                nc.vector.affine_mul_reduce(
                    t1, t1a, s1[:, 128:256], s1[:, 0:128], 1.0, -0.5
                )
                if t == 0:
                    nc.vector.tensor_copy(c_new, t1)
                else:
                    nc.vector.tensor_add(c_new, t1, t2)
            else:
                t1 = tmp_pool.tile([128, 128], sdt, tag=f"t1{pair}", name=f"t1{pair}")
                nc.vector.scalar_tensor_tensor(
                    t1, s1[:, 128:256], 0.5, s1[:, 0:128], ALU.subtract, ALU.mult
                )
                if t == 0:
                    nc.vector.tensor_scalar_mul(c_new, t1, 2.0)
                else:
                    t2 = tmp_pool.tile([128, 128], cdt, tag=f"t2{pair}", name=f"t2{pair}")
                    eng = nc.gpsimd if gps_t2 else nc.vector
                    eng.tensor_mul(t2, s1[:, 256:384], c_prev[pair])
                    nc.vector.scalar_tensor_tensor(
                        c_new, t1, 2.0, t2, ALU.mult, ALU.add
                    )
            c_prev[pair] = c_new

        # chunked y drain overlapped with the loop (DMA cannot read PSUM:
        # copy each finished 64-col block to sbuf off-chain, then DMA it).
        # The y columns for steps [32b, 32(b+1)) are complete once the y
        # matmul for t-1 = 32(b+1)-1 has been emitted (iteration 32(b+1)+1).
        if y_d is not None and t in (17, 33, 49, 65, 81, 97, 113, 123):
            y_dr, y_sb = y_d
            blk = (t - 17) // 16
            sl = slice(32 * blk, 32 * (blk + 1)) if t < 123 else slice(224, 244)
            ps_v = y_ps.rearrange("p (q c) -> p q c", c=2 * T)[:, :, sl]
            sb_v = y_sb.rearrange("p (q c) -> p q c", c=2 * T)[:, :, sl]
            dr_v = y_dr.rearrange("p (q c) -> p q c", c=2 * T)[:, :, sl]
            nc.vector.tensor_copy(sb_v, ps_v)
            nc.sync.dma_start(out=dr_v, in_=sb_v)

        pgas_cur, pgb_cur = pgas_nxt, pgb_nxt

    for pair in range(NPAIR):
        out_sl, h_tile = y_mm[pair]
        nc.tensor.matmul(
            y_ps[:, out_sl], h_tile, outw[:, 2 * pair : 2 * pair + 2],
            start=True, stop=True,
        )
    if y_d is not None:
        y_dr, y_sb = y_d
        sl = slice(244, 256)
        ps_v = y_ps.rearrange("p (q c) -> p q c", c=2 * T)[:, :, sl]
        sb_v = y_sb.rearrange("p (q c) -> p q c", c=2 * T)[:, :, sl]
        dr_v = y_dr.rearrange("p (q c) -> p q c", c=2 * T)[:, :, sl]
        nc.vector.tensor_copy(sb_v, ps_v)
        nc.sync.dma_start(out=dr_v, in_=sb_v)


def _emit_v12(*args, **kw):
    _emit_v11(*args, merge_o=True, **kw)


def _emit_v13(*args, **kw):
    _emit_v11(*args, merge_o=True, gps_t2=True, **kw)


def _emit_v14(*args, **kw):
    _emit_v11(*args, merge_o=True, tanh_cell=True, **kw)


def _emit_v15(*args, **kw):
    _emit_v11(*args, merge_o=True, tanh_cell=True, **kw)


def _emit_v16(nc, tc, mybir, pga_pool, pgb_pool, sig_pool, tan_pool, tmp_pool,
              c_pool, h_pool, xmT, ipw, recw, outw, y_ps, f32, sdt, cdt,
              SIG, TANH, y_d=None):
    """v15 but with the o gate folded into pga: one [i,g,f,o] psum bank per
    pair (512 f32 = exactly one bank), one sigmoid per pair covering all
    four gates, no sigma2 op.  Trades +1 rec matmul and +128 sigmoid cols
    on the chain against removing the off-chain sigma2 from the ACT queue."""
    ALU = mybir.AluOpType
    h_prev = [None] * NPAIR
    c_prev = [None] * NPAIR
    o_src = [None] * NPAIR
    y_mm = [None] * NPAIR

    def alloc_pga(t):
        return [pga_pool.tile([128, 4 * 128], f32, tag=f"pga{p}",
                              name=f"pga{p}_{t}") for p in range(NPAIR)]

    def ip_mms(pgas, t, close):
        xm_t = xmT[:, t * B : (t + 1) * B]
        for pair in range(NPAIR):
            for k in range(4):
                ci = pair * 4 + k
                nc.tensor.matmul(
                    pgas[pair][:, k * 128 : (k + 1) * 128],
                    ipw[:, ci * 128 : (ci + 1) * 128], xm_t,
                    start=(k == 0), stop=(close and k == 3),
                )

    pgas_cur = alloc_pga(0)
    ip_mms(pgas_cur, 0, close=True)

    for t in range(T + 1):
        if t > 0:
            scs = []
            for pair in range(NPAIR):
                sc = tan_pool.tile([128, 128], sdt, tag=f"sc{pair}",
                                   name=f"sc{pair}")
                nc.scalar.activation(sc, c_prev[pair], TANH, scale=2.0)
                scs.append(sc)
            for pair in range(NPAIR):
                h2 = h_pool.tile([128, B], mybir.dt.bfloat16, tag=f"h{pair}",
                                 name=f"h{pair}")
                nc.vector.tensor_mul(h2, scs[pair], o_src[pair])
                h_prev[pair] = h2
                y_mm[pair] = (
                    slice(pair * 2 * T + 2 * (t - 1), pair * 2 * T + 2 * (t - 1) + 2),
                    h2,
                )
        if t == T:
            break

        if t > 0:
            for pair in range(NPAIR):
                for k in range(4):
                    ci = pair * 4 + k
                    nc.tensor.matmul(
                        pgas_cur[pair][:, k * 128 : (k + 1) * 128],
                        recw[:, ci * 128 : (ci + 1) * 128], h_prev[pair],
                        start=False, stop=(k == 3),
                    )
        for pair in range(NPAIR):
            if y_mm[pair] is not None:
                out_sl, h_tile = y_mm[pair]
                nc.tensor.matmul(
                    y_ps[:, out_sl], h_tile, outw[:, 2 * pair : 2 * pair + 2],
                    start=True, stop=True,
                )
                y_mm[pair] = None
        if t + 1 < T:
            pgas_nxt = alloc_pga(t + 1)
            ip_mms(pgas_nxt, t + 1, close=False)
        else:
            pgas_nxt = None

        s1s = []
        for pair in range(NPAIR):
            s1 = sig_pool.tile([128, 4 * 128], sdt, tag=f"s1{pair}", name=f"s1{pair}")
            nc.scalar.activation(s1, pgas_cur[pair], SIG)
            s1s.append(s1)
            o_src[pair] = s1[:, 384:512]

        for pair in range(NPAIR):
            s1 = s1s[pair]
            c_new = c_pool.tile([128, 128], cdt, tag=f"c{pair}", name=f"c{pair}")
            if t > 0:
                t2 = tmp_pool.tile([128, 128], cdt, tag=f"t2{pair}", name=f"t2{pair}")
                nc.vector.tensor_mul(t2, s1[:, 256:384], c_prev[pair])
            t1 = tmp_pool.tile([128, 128], sdt, tag=f"t1{pair}", name=f"t1{pair}")
            nc.vector.scalar_tensor_tensor(
                t1, s1[:, 128:256], 0.5, s1[:, 0:128], ALU.subtract, ALU.mult
            )
            if t == 0:
                nc.vector.tensor_copy(c_new, t1)
            else:
                nc.vector.tensor_add(c_new, t1, t2)
            c_prev[pair] = c_new

        if y_d is not None and t in (17, 33, 49, 65, 81, 97, 113, 123):
            y_dr, y_sb = y_d
            blk = (t - 17) // 16
            sl = slice(32 * blk, 32 * (blk + 1)) if t < 123 else slice(224, 244)
            ps_v = y_ps.rearrange("p (q c) -> p q c", c=2 * T)[:, :, sl]
            sb_v = y_sb.rearrange("p (q c) -> p q c", c=2 * T)[:, :, sl]
            dr_v = y_dr.rearrange("p (q c) -> p q c", c=2 * T)[:, :, sl]
            nc.vector.tensor_copy(sb_v, ps_v)
            nc.sync.dma_start(out=dr_v, in_=sb_v)

        pgas_cur = pgas_nxt

    for pair in range(NPAIR):
        out_sl, h_tile = y_mm[pair]
        nc.tensor.matmul(
            y_ps[:, out_sl], h_tile, outw[:, 2 * pair : 2 * pair + 2],
            start=True, stop=True,
        )
    if y_d is not None:
        y_dr, y_sb = y_d
        sl = slice(244, 256)
        ps_v = y_ps.rearrange("p (q c) -> p q c", c=2 * T)[:, :, sl]
        sb_v = y_sb.rearrange("p (q c) -> p q c", c=2 * T)[:, :, sl]
        dr_v = y_dr.rearrange("p (q c) -> p q c", c=2 * T)[:, :, sl]
        nc.vector.tensor_copy(sb_v, ps_v)
        nc.sync.dma_start(out=dr_v, in_=sb_v)


def _emit_v9(nc, tc, mybir, pga_pool, pgb_pool, sig_pool, tan_pool, tmp_pool,
             c_pool, h_pool, xmT, ipw, recw, outw, y_ps, f32, sdt, cdt,
             SIG, TANH):
    """v8 with the critical sigmoid shrunk to [i,g]: pga=[i,g] (bufs=2,
    sigma1 waits only 4 matmuls), pgfo=[f,o] (bufs=1, one off-chain
    sigmoid covers both f for t2 and o for next step's h2).  Total ACT
    busy is unchanged vs v8; the chain segments shrink."""
    ALU = mybir.AluOpType
    h_prev = [None] * NPAIR
    c_prev = [None] * NPAIR
    o_src = [None] * NPAIR
    y_mm = [None] * NPAIR

    for t in range(T + 1):
        if t > 0:
            scs = []
            for pair in range(NPAIR):
                sc = tan_pool.tile([128, 128], f32, tag=f"sc{pair}", name=f"sc{pair}")
                nc.scalar.activation(sc, c_prev[pair], SIG, scale=2.0)
                scs.append(sc)
            for pair in range(NPAIR):
                h2 = h_pool.tile([128, B], mybir.dt.bfloat16, tag=f"h{pair}",
                                 name=f"h{pair}")
                nc.vector.scalar_tensor_tensor(
                    h2, scs[pair], 0.5, o_src[pair], ALU.subtract, ALU.mult
                )
                h_prev[pair] = h2
                y_mm[pair] = (
                    slice(pair * 2 * T + 2 * (t - 1), pair * 2 * T + 2 * (t - 1) + 2),
                    h2,
                )
        if t == T:
            break

        xm_t = xmT[:, t * B : (t + 1) * B]

        # [i, g] chunks into pga (both pairs), then [f, o] into pgfo
        pgas, pgfos = [], []
        for pair in range(NPAIR):
            pga = pga_pool.tile([128, 2 * 128], f32, tag=f"pga{pair}",
                                name=f"pga{pair}")
            pgas.append(pga)
            for k in range(2):
                ci = pair * 4 + k
                sl = slice(k * 128, (k + 1) * 128)
                wsl = slice(ci * 128, (ci + 1) * 128)
                if t > 0:
                    nc.tensor.matmul(pga[:, sl], recw[:, wsl], h_prev[pair],
                                     start=True, stop=False)
                nc.tensor.matmul(pga[:, sl], ipw[:, wsl], xm_t,
                                 start=(t == 0), stop=True)
        for pair in range(NPAIR):
            pgfo = pgb_pool.tile([128, 2 * 128], f32, tag=f"pgfo{pair}",
                                 name=f"pgfo{pair}")
            pgfos.append(pgfo)
            for k in range(2, 4):
                ci = pair * 4 + k
                sl = slice((k - 2) * 128, (k - 1) * 128)
                wsl = slice(ci * 128, (ci + 1) * 128)
                if t > 0:
                    nc.tensor.matmul(pgfo[:, sl], recw[:, wsl], h_prev[pair],
                                     start=True, stop=False)
                nc.tensor.matmul(pgfo[:, sl], ipw[:, wsl], xm_t,
                                 start=(t == 0), stop=True)
        for pair in range(NPAIR):
            if y_mm[pair] is not None:
                out_sl, h_tile = y_mm[pair]
                nc.tensor.matmul(
                    y_ps[:, out_sl], h_tile, outw[:, 2 * pair : 2 * pair + 2],
                    start=True, stop=True,
                )
                y_mm[pair] = None

        # per pair: sigma1=[i,g] (chain), sigma_fo=[f,o] (feeds t2 and
        # next step's h2) -- interleaved so each pair's f arrives early
        s1s, sfos = [], []
        for pair in range(NPAIR):
            s1 = sig_pool.tile([128, 2 * 128], sdt, tag=f"s1{pair}", name=f"s1{pair}")
            nc.scalar.activation(s1, pgas[pair], SIG)
            s1s.append(s1)
            sfo = sig_pool.tile([128, 2 * 128], sdt, tag=f"sfo{pair}",
                                name=f"sfo{pair}")
            nc.scalar.activation(sfo, pgfos[pair], SIG)
            sfos.append(sfo)
            o_src[pair] = sfo[:, 128:256]

        for pair in range(NPAIR):
            s1 = s1s[pair]
            t1 = tmp_pool.tile([128, 128], sdt, tag=f"t1{pair}", name=f"t1{pair}")
            nc.vector.scalar_tensor_tensor(
                t1, s1[:, 128:256], 0.5, s1[:, 0:128], ALU.subtract, ALU.mult
            )
            c_new = c_pool.tile([128, 128], cdt, tag=f"c{pair}", name=f"c{pair}")
            if t == 0:
                nc.vector.tensor_scalar_mul(c_new, t1, 2.0)
            else:
                t2 = tmp_pool.tile([128, 128], cdt, tag=f"t2{pair}", name=f"t2{pair}")
                nc.vector.tensor_mul(t2, sfos[pair][:, 0:128], c_prev[pair])
                nc.vector.scalar_tensor_tensor(
                    c_new, t1, 2.0, t2, ALU.mult, ALU.add
                )
            c_prev[pair] = c_new

    for pair in range(NPAIR):
        out_sl, h_tile = y_mm[pair]
        nc.tensor.matmul(
            y_ps[:, out_sl], h_tile, outw[:, 2 * pair : 2 * pair + 2],
            start=True, stop=True,
        )


def _emit_v10(nc, tc, mybir, pga_pool, pgb_pool, sig_pool, tan_pool, tmp_pool,
              c_pool, h_pool, xmT, ipw, recw, outw, y_ps, f32, sdt, cdt,
              SIG, TANH):
    """Two ACT instructions per pair per step (vs 3):
      pga = [i,g,f] (bufs=2): sigma1 = sigmoid(pga), feeds t1 and t2.
      pgo = [o_half | c] (bufs=1): the o matmuls write o/2 logits (weights
        halved on the host), the DVE cell STT writes c into the spare half
        of the same psum bank; one sigmoid with scale=2 then yields
        [sigma(o) | sigma(2c)] for the h2 STT.
    c lives in PSUM; t2 reads it there (1x + psum init, still a win)."""
    ALU = mybir.AluOpType
    h_prev = [None] * NPAIR
    c_prev = [None] * NPAIR  # psum AP of c inside pgo tile
    so_sc = [None] * NPAIR  # sigmoid output [s_o | s_c]
    y_mm = [None] * NPAIR

    for t in range(T + 1):
        # ---- tail of step t-1: sigma over [o|c], then h2 ----
        if t > 0:
            for pair in range(NPAIR):
                so = sig_pool.tile([128, 2 * 128], sdt if False else f32,
                                   tag=f"so{pair}", name=f"so{pair}")
                nc.scalar.activation(so, so_sc[pair], SIG, scale=2.0)
                so_sc[pair] = so
            for pair in range(NPAIR):
                so = so_sc[pair]
                h2 = h_pool.tile([128, B], mybir.dt.bfloat16, tag=f"h{pair}",
                                 name=f"h{pair}")
                nc.vector.scalar_tensor_tensor(
                    h2, so[:, 128:256], 0.5, so[:, 0:128], ALU.subtract, ALU.mult
                )
                h_prev[pair] = h2
                y_mm[pair] = (
                    slice(pair * 2 * T + 2 * (t - 1), pair * 2 * T + 2 * (t - 1) + 2),
                    h2,
                )
        if t == T:
            break

        xm_t = xmT[:, t * B : (t + 1) * B]

        # [i, g, f] chunks into pga; [o] into pgo (c joins later)
        pgas, pgos = [], []
        for pair in range(NPAIR):
            pga = pga_pool.tile([128, 3 * 128], f32, tag=f"pga{pair}",
                                name=f"pga{pair}")
            pgas.append(pga)
            for k in range(3):
                ci = pair * 4 + k
                sl = slice(k * 128, (k + 1) * 128)
                wsl = slice(ci * 128, (ci + 1) * 128)
                if t > 0:
                    nc.tensor.matmul(pga[:, sl], recw[:, wsl], h_prev[pair],
                                     start=True, stop=False)
                nc.tensor.matmul(pga[:, sl], ipw[:, wsl], xm_t,
                                 start=(t == 0), stop=True)
        for pair in range(NPAIR):
            pgo = pgb_pool.tile([128, 2 * 128], f32, tag=f"pgo{pair}",
                                name=f"pgo{pair}")
            pgos.append(pgo)
            ci = pair * 4 + 3
            wsl = slice(ci * 128, (ci + 1) * 128)
            if t > 0:
                nc.tensor.matmul(pgo[:, 0:128], recw[:, wsl], h_prev[pair],
                                 start=True, stop=False)
            nc.tensor.matmul(pgo[:, 0:128], ipw[:, wsl], xm_t,
                             start=(t == 0), stop=True)
        for pair in range(NPAIR):
            if y_mm[pair] is not None:
                out_sl, h_tile = y_mm[pair]
                nc.tensor.matmul(
                    y_ps[:, out_sl], h_tile, outw[:, 2 * pair : 2 * pair + 2],
                    start=True, stop=True,
                )
                y_mm[pair] = None

        # sigma1 over [i, g, f]
        s1s = []
        for pair in range(NPAIR):
            s1 = sig_pool.tile([128, 3 * 128], sdt, tag=f"s1{pair}", name=f"s1{pair}")
            nc.scalar.activation(s1, pgas[pair], SIG)
            s1s.append(s1)

        # cell update; c written into pgo[:, 128:256] (psum)
        for pair in range(NPAIR):
            s1 = s1s[pair]
            c_ap = pgos[pair][:, 128:256]
            t1 = tmp_pool.tile([128, 128], sdt, tag=f"t1{pair}", name=f"t1{pair}")
            nc.vector.scalar_tensor_tensor(
                t1, s1[:, 128:256], 0.5, s1[:, 0:128], ALU.subtract, ALU.mult
            )
            if t == 0:
                nc.vector.tensor_scalar_mul(c_ap, t1, 2.0)
            else:
                t2 = tmp_pool.tile([128, 128], f32, tag=f"t2{pair}", name=f"t2{pair}")
                nc.vector.tensor_mul(t2, s1[:, 256:384], c_prev[pair])
                nc.vector.scalar_tensor_tensor(
                    c_ap, t1, 2.0, t2, ALU.mult, ALU.add
                )
            c_prev[pair] = c_ap
            so_sc[pair] = pgos[pair]  # [o_half | c] for the next tail sigma

    for pair in range(NPAIR):
        out_sl, h_tile = y_mm[pair]
        nc.tensor.matmul(
            y_ps[:, out_sl], h_tile, outw[:, 2 * pair : 2 * pair + 2],
            start=True, stop=True,
        )


def _emit_v6(nc, tc, mybir, pga_pool, pgb_pool, sig_pool, tan_pool, tmp_pool,
             c_pool, h_pool, xmT, ipw, recw, outw, y_ps, f32, sdt, cdt,
             SIG, TANH):
    """v5 + two-bank psum split per pair: pga=[i,g] (bufs=2), pgb=[f,o]
    (bufs=1), so sigma1 only waits on 4 matmuls and is smaller."""
    ALU = mybir.AluOpType
    h_prev = [None] * NPAIR
    c_prev = [None] * NPAIR
    o_src = [None] * NPAIR
    y_mm = [None] * NPAIR

    for t in range(T + 1):
        if t > 0:
            scs = []
            for pair in range(NPAIR):
                sc = tan_pool.tile([128, 128], f32, tag=f"sc{pair}", name=f"sc{pair}")
                nc.scalar.activation(sc, c_prev[pair], SIG, scale=2.0)
                scs.append(sc)
            for pair in range(NPAIR):
                h2 = h_pool.tile([128, B], mybir.dt.bfloat16, tag=f"h{pair}",
                                 name=f"h{pair}")
                nc.vector.scalar_tensor_tensor(
                    h2, scs[pair], 0.5, o_src[pair], ALU.subtract, ALU.mult
                )
                h_prev[pair] = h2
                y_mm[pair] = (
                    slice(pair * 2 * T + 2 * (t - 1), pair * 2 * T + 2 * (t - 1) + 2),
                    h2,
                )
        if t == T:
            break

        xm_t = xmT[:, t * B : (t + 1) * B]

        # gate matmuls: [i,g] chunks (pga) for both pairs first, then [f,o]
        pgas, pgbs = [], []
        for pair in range(NPAIR):
            pga = pga_pool.tile([128, 2 * 128], f32, tag=f"pga{pair}",
                                name=f"pga{pair}")
            pgas.append(pga)
            for k in range(2):
                ci = pair * 4 + k
                sl = slice(k * 128, (k + 1) * 128)
                wsl = slice(ci * 128, (ci + 1) * 128)
                if t > 0:
                    nc.tensor.matmul(pga[:, sl], recw[:, wsl], h_prev[pair],
                                     start=True, stop=False)
                nc.tensor.matmul(pga[:, sl], ipw[:, wsl], xm_t,
                                 start=(t == 0), stop=True)
        for pair in range(NPAIR):
            pgb = pgb_pool.tile([128, 2 * 128], f32, tag=f"pgb{pair}",
                                name=f"pgb{pair}")
            pgbs.append(pgb)
            for k in range(2, 4):
                ci = pair * 4 + k
                sl = slice((k - 2) * 128, (k - 1) * 128)
                wsl = slice(ci * 128, (ci + 1) * 128)
                if t > 0:
                    nc.tensor.matmul(pgb[:, sl], recw[:, wsl], h_prev[pair],
                                     start=True, stop=False)
                nc.tensor.matmul(pgb[:, sl], ipw[:, wsl], xm_t,
                                 start=(t == 0), stop=True)
        for pair in range(NPAIR):
            if y_mm[pair] is not None:
                out_sl, h_tile = y_mm[pair]
                nc.tensor.matmul(
                    y_ps[:, out_sl], h_tile, outw[:, 2 * pair : 2 * pair + 2],
                    start=True, stop=True,
                )
                y_mm[pair] = None

        # sigma1 = [i, g]; sigma2 = [f, o]
        s1s, s2s = [], []
        for pair in range(NPAIR):
            s1 = sig_pool.tile([128, 2 * 128], sdt, tag=f"s1{pair}",
                               name=f"s1{pair}")
            nc.scalar.activation(s1, pgas[pair], SIG)
            s1s.append(s1)
        for pair in range(NPAIR):
            s2 = sig_pool.tile([128, 2 * 128], sdt, tag=f"s2{pair}",
                               name=f"s2{pair}")
            nc.scalar.activation(s2, pgbs[pair], SIG)
            s2s.append(s2)
            o_src[pair] = s2[:, 128:256]

        # t1' = (s_g - 0.5)*s_i ; c = 2*t1' + s_f*c_prev
        for pair in range(NPAIR):
            s1 = s1s[pair]
            t1 = tmp_pool.tile([128, 128], sdt, tag=f"t1{pair}", name=f"t1{pair}")
            nc.vector.scalar_tensor_tensor(
                t1, s1[:, 128:256], 0.5, s1[:, 0:128], ALU.subtract, ALU.mult
            )
            c_new = c_pool.tile([128, 128], cdt, tag=f"c{pair}", name=f"c{pair}")
            if t == 0:
                nc.vector.tensor_scalar_mul(c_new, t1, 2.0)
            else:
                t2 = tmp_pool.tile([128, 128], cdt, tag=f"t2{pair}", name=f"t2{pair}")
                nc.vector.tensor_mul(t2, s2s[pair][:, 0:128], c_prev[pair])
                nc.vector.scalar_tensor_tensor(
                    c_new, t1, 2.0, t2, ALU.mult, ALU.add
                )
            c_prev[pair] = c_new

    for pair in range(NPAIR):
        out_sl, h_tile = y_mm[pair]
        nc.tensor.matmul(
            y_ps[:, out_sl], h_tile, outw[:, 2 * pair : 2 * pair + 2],
            start=True, stop=True,
        )


def _w_full(W_ih_d, d):
    """[4H, D-1] -> [4H, D] with column d zero and the 'other feature'
    columns scattered back to their true feature index."""
    out = np.zeros((4 * H, D), np.float32)
    idx = [j for j in range(D) if j != d]
    out[:, idx] = W_ih_d
    return out


def _pack_core_inputs(core, xmT_np, W_ih, W_hh, b_ih, b_hh, W_out):
    """Pack weights for one core (features 4*core .. 4*core+3)."""
    ipw = np.zeros((D + 1, 8 * 128), np.float32)
    recw = np.zeros((128, 8 * 128), np.float32)
    outw = np.zeros((128, 2 * NPAIR), np.float32)

    for ci, (gate, pair) in enumerate(CHUNK_DEFS):
        gs = GATE_SLICES[gate]
        for half in range(2):
            d = DLOC * core + 2 * pair + half
            cols = slice(ci * 128 + 64 * half, ci * 128 + 64 * half + 64)
            rows = slice(64 * half, 64 * half + 64)
            # rec: block-diag W_hh[d, gate_rows, :].T  ([K=h, M=gate_row])
            recw[rows, cols] = W_hh[d, gs, :].T
            # ip: full-D input weights with zero self-column, bias in row 32
            wf = _w_full(W_ih[d], d)  # [4H, D]
            ipw[0:D, cols] = wf[gs, :].T
            ipw[D, cols] = b_ih[d, gs] + b_hh[d, gs]

    for pair in range(NPAIR):
        for half in range(2):
            d = DLOC * core + 2 * pair + half
            outw[64 * half : 64 * half + 64, 2 * pair + half] = W_out[d]

    if V3:
        # sigma-trick folding: h is stored as h/2 -> double recw/outw;
        # g-gate logits doubled -> double g chunks of ipw and recw again.
        recw *= 2.0
        outw *= 2.0
        for ci, (gate, _) in enumerate(CHUNK_DEFS):
            if gate == "g":
                ipw[:, ci * 128 : (ci + 1) * 128] *= 2.0
                recw[:, ci * 128 : (ci + 1) * 128] *= 2.0
            if gate == "o" and VARIANT == "v10":
                # o logits halved so one sigmoid(scale=2) over [o|c]
                # yields sigma(o) and sigma(2c) together
                ipw[:, ci * 128 : (ci + 1) * 128] *= 0.5
                recw[:, ci * 128 : (ci + 1) * 128] *= 0.5
    elif TANH_CELL:
        # h stored full-scale; only the g-gate sigma-trick doubling remains
        for ci, (gate, _) in enumerate(CHUNK_DEFS):
            if gate == "g":
                ipw[:, ci * 128 : (ci + 1) * 128] *= 2.0
                recw[:, ci * 128 : (ci + 1) * 128] *= 2.0

    return {
        "xmT": xmT_np,
        "ipw": ipw.astype(BF16),
        "recw": recw.astype(BF16),
        "outw": outw.astype(BF16),
    }


def _prep_in_maps(x_raw, mask_pad, W_ih, W_hh, b_ih, b_hh, W_out):
    xm = np.where(mask_pad[:, :, None], x_raw, 0.0).astype(np.float32)  # [B,T,D]
    xmT = np.empty((D + 1, T * B), np.float32)
    xmT[0:D] = xm.transpose(2, 1, 0).reshape(D, T * B)  # [d, t*B + b]
    xmT[D] = 1.0
    xmT_np = xmT.astype(BF16)
    return [
        _pack_core_inputs(k, xmT_np, W_ih, W_hh, b_ih, b_hh, W_out)
        for k in range(NCORES)
    ]


def _assemble_output(results, b_out):
    """results[k]["y"]: [B, NPAIR*2*T] fp32, layout [b, pair*2T + t*2 + half]."""
    x_hat = np.empty((B, T, D), np.float32)
    for k in range(NCORES):
        y = np.asarray(results[k]["y"]).reshape(B, NPAIR, T, 2)
        for pair in range(NPAIR):
            for half in range(2):
                d = DLOC * k + 2 * pair + half
                x_hat[:, :, d] = y[:, pair, :, half] + b_out[d]
    return x_hat


def kernel(x_raw, mask_pad, W_ih, W_hh, b_ih, b_hh, W_out, b_out):
    x_raw = np.asarray(x_raw, np.float32)
    mask_pad = np.asarray(mask_pad)
    W_ih = np.asarray(W_ih, np.float32)
    W_hh = np.asarray(W_hh, np.float32)
    b_ih = np.asarray(b_ih, np.float32)
    b_hh = np.asarray(b_hh, np.float32)
    W_out = np.asarray(W_out, np.float32)
    b_out = np.asarray(b_out, np.float32)

    from concourse import bass_utils

    nc = _build_bass()
    in_maps = _prep_in_maps(x_raw, mask_pad, W_ih, W_hh, b_ih, b_hh, W_out)
    res = bass_utils.run_bass_kernel_spmd(
        nc,
        in_maps,
        core_ids=list(range(NCORES)),
        trace=bool(int(os.environ.get("KERNEL_TRACE", "0"))),
    )
    _CACHE["last_results"] = res
    return _assemble_output(res.results, b_out)

